# revision 22
# baseline (speedup 1.0000x reference)
# Correlation2D (RAFT-style correlation pyramid lookup) on 8 TRN2 NeuronCores.
#
# Sharding: data-parallel over the bs*h*w query axis. Each core owns 1024
# queries (= 8 image rows). Per 128-query block it computes its slice of the
# cost volume via bf16 GEMM (fmap2 replicated, pyramid pooling folded into
# fmap2), stores the 4-level pyramid per-query-contiguous to a per-block DRAM
# buffer in bf16, then gathers ONE long contiguous run per (query, level)
# covering the whole 10-row patch footprint (9*Wl+10 elements) with a single
# indirect DMA per (block, level). The separable bilinear combine reads the
# patch taps out of the run via strided overlapping views. Blocks are
# software-pipelined (combine of block b-1 overlaps GEMM of block b).
import numpy as np

# ---- problem constants (hardcoded per contest contract) ----
H, W = 64, 128
D = 256
NUM_LEVELS = 4
RADIUS = 4
KK = 2 * RADIUS + 1        # 9
PS = KK + 1                # 10x10 patch per (query, level)
NCORES = 8
QPC = (H * W) // NCORES    # 1024 queries per core
NBLK = QPC // 128          # 8 blocks of 128 queries
LVL_W = [W >> l for l in range(NUM_LEVELS)]            # 128 64 32 16
LVL_H = [H >> l for l in range(NUM_LEVELS)]            # 64 32 16 8
LVL_N = [LVL_W[l] * LVL_H[l] for l in range(NUM_LEVELS)]   # 8192 2048 512 128
LVL_OFF = [sum(LVL_N[:l]) for l in range(NUM_LEVELS)]  # 0 8192 10240 10752
LVLSUM = sum(LVL_N)        # 10880
RUN_L = [(PS - 1) * LVL_W[l] + PS for l in range(NUM_LEVELS)]  # 1162 586 298 154
PADB = 1024                # zeroed head/tail pad (elements) of each block buffer
NTOTB = PADB + 128 * LVLSUM + PADB  # 1394688 = 1362 * 1024
NCH = NUM_LEVELS * KK * KK  # 324 output channels
MM_N = 512                 # matmul N-chunk (one PSUM bank of f32)

_CACHE = {}


def _emit(ctx, tc, out_ext, f1c, f2, crd, dbg=None):
    import concourse.bass as bass
    import concourse.mybir as mybir
    from concourse.masks import make_identity

    nc = tc.nc
    f32 = mybir.dt.float32
    bf16 = mybir.dt.bfloat16
    i32 = mybir.dt.int32
    Alu = mybir.AluOpType

    const_pool = ctx.enter_context(tc.tile_pool(name="constp", bufs=1))
    f2_pool = ctx.enter_context(tc.tile_pool(name="f2p", bufs=1))
    coordp = ctx.enter_context(tc.tile_pool(name="coordp", bufs=1))
    small = ctx.enter_context(tc.tile_pool(name="small", bufs=2))
    lhsp = ctx.enter_context(tc.tile_pool(name="lhsp", bufs=2))
    cvsb = ctx.enter_context(tc.tile_pool(name="cvsb", bufs=2))
    runp = ctx.enter_context(tc.tile_pool(name="runp", bufs=3))
    cmb = ctx.enter_context(tc.tile_pool(name="cmb", bufs=2))
    outp = ctx.enter_context(tc.tile_pool(name="outp", bufs=1))
    psum = ctx.enter_context(tc.tile_pool(name="psum", bufs=3, space="PSUM"))
    psum_t = ctx.enter_context(tc.tile_pool(name="psumt", bufs=2, space="PSUM"))
    dramp = ctx.enter_context(tc.tile_pool(name="dramp", bufs=1, space="DRAM"))

    # ---------------- per-block DRAM cv buffers (bf16) ---------------------
    cvb = [dramp.tile([NTOTB], bf16, name=f"cvb{b}") for b in range(NBLK)]

    # zero head/tail pads (OOB run spill lands there; must stay finite since
    # it is multiplied by a zero weight)
    # ---------------- coords -> run indices / interp weights ----------------
    # cx/cy for this core's queries: [128, 8]  (partition p = x, free b = y row)
    cxs = coordp.tile([128, NBLK], f32, name="cxs")
    cys = coordp.tile([128, NBLK], f32, name="cys")
    nc.scalar.dma_start(out=cxs[:], in_=crd[0, :].rearrange("(b p) -> p b", p=128))
    nc.scalar.dma_start(out=cys[:], in_=crd[1, :].rearrange("(b p) -> p b", p=128))

    # lhs prefetch: block 0 loads are posted before the (big) f2h1 chunks on
    # the scalar queue so the first GEMM group is not starved
    lhs_cache = {}

    def load_lhs(b):
        tiles = []
        for k in range(2):
            lt = lhsp.tile([128, 128], bf16, name=f"lhsT{k}", tag=f"lhsT{k}")
            nc.scalar.dma_start(
                out=lt[:], in_=f1c[k * 128 : (k + 1) * 128, b * 128 : (b + 1) * 128]
            )
            tiles.append(lt)
        lhs_cache[b] = tiles

    load_lhs(0)

    # per-query element base offset within its block buffer: PADB + p*LVLSUM
    bq_i = coordp.tile([128, NBLK], i32, name="bq_i")
    nc.gpsimd.iota(bq_i[:], pattern=[[0, NBLK]], base=PADB, channel_multiplier=LVLSUM)
    bqf = coordp.tile([128, NBLK], f32, name="bqf")
    nc.vector.tensor_copy(out=bqf[:], in_=bq_i[:])

    idx_i = coordp.tile([128, NBLK, NUM_LEVELS], i32, name="idx_i")
    wx0e = coordp.tile([128, NBLK, NUM_LEVELS, KK], f32, name="wx0e")
    wx1e = coordp.tile([128, NBLK, NUM_LEVELS, KK], f32, name="wx1e")
    wy0e = coordp.tile([128, NBLK, NUM_LEVELS, KK], f32, name="wy0e")
    wy1e = coordp.tile([128, NBLK, NUM_LEVELS, KK], f32, name="wy1e")

    # c ramp: -4..5 (patch tap -> offset from floor(coord))
    cramp_i = const_pool.tile([128, PS], i32, name="cramp_i")
    nc.gpsimd.iota(cramp_i[:], pattern=[[1, PS]], base=-RADIUS, channel_multiplier=0)
    crampf = const_pool.tile([128, PS], f32, name="crampf")
    nc.vector.tensor_copy(out=crampf[:], in_=cramp_i[:])

    for l in range(NUM_LEVELS):
        Wl, Hl = LVL_W[l], LVL_H[l]
        inv = 1.0 / (1 << l)
        # floor + frac (tiny [128,8] ops; vector)
        flo = {}
        fra = {}
        for nm, src in (("x", cxs), ("y", cys)):
            xs = small.tile([128, NBLK], f32, name=f"xs{nm}{l}", tag="xs")
            nc.vector.tensor_scalar_mul(xs[:], src[:], inv)
            ii = small.tile([128, NBLK], i32, name=f"ii{nm}{l}", tag="ii")
            nc.vector.tensor_copy(out=ii[:], in_=xs[:])
            ff = small.tile([128, NBLK], f32, name=f"ff{nm}{l}", tag="ff")
            nc.vector.tensor_copy(out=ff[:], in_=ii[:])
            # f32->i32 cast rounds to nearest; correct round-ups back down
            adj = small.tile([128, NBLK], f32, name=f"adj{nm}{l}", tag="adj")
            nc.vector.tensor_tensor(adj[:], ff[:], xs[:], op=Alu.is_gt)
            nc.vector.tensor_tensor(ff[:], ff[:], adj[:], op=Alu.subtract)
            fr = small.tile([128, NBLK], f32, name=f"fr{nm}{l}", tag="fr")
            nc.vector.tensor_tensor(fr[:], xs[:], ff[:], op=Alu.subtract)
            flo[nm], fra[nm] = ff, fr

        # interp weights with the OOB zero-mask folded in
        for (w0t, w1t), nm, lim in (
            ((wx0e, wx1e), "x", Wl - 1),
            ((wy0e, wy1e), "y", Hl - 1),
        ):
            posf, frac = flo[nm], fra[nm]
            pos = small.tile([128, NBLK, PS], f32, name=f"pos{nm}{l}", tag="pos")
            nc.gpsimd.tensor_tensor(
                pos[:],
                posf[:].unsqueeze(2).to_broadcast([128, NBLK, PS]),
                crampf[:].unsqueeze(1).to_broadcast([128, NBLK, PS]),
                op=Alu.add,
            )
            oka = small.tile([128, NBLK, PS], f32, name=f"oka{nm}{l}", tag="oka")
            nc.gpsimd.tensor_scalar(pos[:], pos[:], -float(lim) / 2.0, 2.0,
                                    op0=Alu.add, op1=Alu.mult)
            # pos' = 2*p - lim; in-bounds <=> |pos'| <= lim <=> pos'^2 <= lim^2
            # <=> min(max(lim^2 + 1 - pos'^2, 0), 1) == 1  (all-integer values)
            nc.gpsimd.tensor_tensor(oka[:], pos[:], pos[:], op=Alu.mult)
            nc.gpsimd.tensor_scalar(oka[:], oka[:], -1.0,
                                    float(lim) * float(lim) + 1.0,
                                    op0=Alu.mult, op1=Alu.add)
            nc.gpsimd.tensor_scalar(oka[:], oka[:], 0.0, 1.0,
                                    op0=Alu.max, op1=Alu.min)
            w0 = small.tile([128, NBLK], f32, name=f"w0{nm}{l}", tag="w0")
            nc.gpsimd.tensor_scalar(w0[:], frac[:], -1.0, 1.0,
                                    op0=Alu.mult, op1=Alu.add)  # 1 - frac
            nc.gpsimd.tensor_tensor(
                w0t[:, :, l, :],
                w0[:].unsqueeze(2).to_broadcast([128, NBLK, KK]),
                oka[:, :, 0:KK],
                op=Alu.mult,
            )
            nc.gpsimd.tensor_tensor(
                w1t[:, :, l, :],
                frac[:].unsqueeze(2).to_broadcast([128, NBLK, KK]),
                oka[:, :, 1:PS],
                op=Alu.mult,
            )

        # run start index: base + lvl_off + (iy-4)*Wl + (ix-4)
        t1 = small.tile([128, NBLK], f32, name=f"t1{l}", tag="t1")
        nc.vector.tensor_scalar_mul(t1[:], flo["y"][:], float(Wl))
        nc.vector.tensor_tensor(t1[:], t1[:], flo["x"][:], op=Alu.add)
        nc.vector.tensor_tensor(t1[:], t1[:], bqf[:], op=Alu.add)
        nc.vector.tensor_scalar_add(
            t1[:], t1[:], float(LVL_OFF[l] - RADIUS * Wl - RADIUS)
        )
        nc.vector.tensor_copy(out=idx_i[:, :, l], in_=t1[:])  # exact ints

    # ---------------- fmap2 load + pyramid pooling (bf16 sums) -------------
    # f2 as two K-halves [128 chan, 8192 pix]; pooled levels keep raw SUMS,
    # the 1/16 * 0.25^l scale is folded into the PSUM drain.
    f2_lv = []
    halves = []
    for k in range(2):
        f2h = f2_pool.tile([128, LVL_N[0]], bf16, name=f"f2h{k}")
        eng = nc.sync if k == 0 else nc.scalar
        # split so the columns the first GEMM groups need arrive first
        eng.dma_start(
            out=f2h[:, 0:4096], in_=f2[k * 128 : (k + 1) * 128, 0:4096]
        )
        eng.dma_start(
            out=f2h[:, 4096:8192], in_=f2[k * 128 : (k + 1) * 128, 4096:8192]
        )
        halves.append(f2h)
    f2_lv.append(halves)

    # zero the per-block pad regions (after the f2h0 post so the f2 load
    # transfer starts first on the sync queue)
    ztile = const_pool.tile([8, 128], bf16, name="ztile")
    nc.vector.memset(ztile[:], 0.0)
    for b in range(NBLK):
        nc.sync.dma_start(
            out=cvb[b][0:PADB].rearrange("(p x) -> p x", p=8), in_=ztile[:]
        )
        nc.sync.dma_start(
            out=cvb[b][NTOTB - PADB : NTOTB].rearrange("(p x) -> p x", p=8),
            in_=ztile[:],
        )
    for l in range(1, NUM_LEVELS):
        Wl, Hl = LVL_W[l], LVL_H[l]
        pw, ph = LVL_W[l - 1], LVL_H[l - 1]
        halves = []
        for k in range(2):
            eng = nc.vector
            prev = f2_lv[l - 1][k][:].rearrange(
                "p (h w two) -> p h w two", h=ph, w=pw // 2, two=2
            )
            s1 = small.tile(
                [128, ph, pw // 2], bf16, name=f"s1_{l}_{k}", tag=f"poolt{k}", bufs=1
            )
            eng.tensor_tensor(
                s1[:], prev[:, :, :, 0], prev[:, :, :, 1], op=Alu.add
            )
            s1v = s1[:].rearrange("p (h2 two) w -> p h2 two w", h2=Hl, two=2)
            cur = f2_pool.tile([128, Hl * Wl], bf16, name=f"f2l{l}_{k}")
            curv = cur[:].rearrange("p (h w) -> p h w", h=Hl, w=Wl)
            eng.tensor_tensor(
                curv[:], s1v[:, :, 0, :], s1v[:, :, 1, :], op=Alu.add
            )
            halves.append(cur)
        f2_lv.append(halves)

    # chunk schedule: (level, n0, n1) pairs share one 2-bank psum tile
    chunks = []
    for l in range(NUM_LEVELS):
        for n0 in range(0, LVL_N[l], MM_N):
            chunks.append((l, n0, min(LVL_N[l], n0 + MM_N)))
    pairs = [tuple(chunks[i : i + 2]) for i in range(0, len(chunks), 2)]  # 11

    ident = const_pool.tile([128, 128], f32, name="ident")
    make_identity(nc, ident[:])
    OCHUNKS = [(0, 128), (128, 128), (256, NCH - 256)]
    outT = [
        outp.tile([128, NBLK, 128], f32, name=f"outT{k}") for k in range(3)
    ]

    def emit_block_gemm(b):
        lhs = lhs_cache.pop(b)
        if b + 1 < NBLK:
            load_lhs(b + 1)
        cv_sb = cvsb.tile([128, LVLSUM], bf16, name="cv_sb", tag="cv_sb")
        parity = 0
        for g0 in range(0, len(pairs), 3):
            grp = pairs[g0 : g0 + 3]
            pts = []
            for gi, pr in enumerate(grp):
                pts.append(psum.tile([128, 1024], f32, name="pt", tag="pt"))
            # weights-major order: all lhs0 matmuls, then all lhs1
            for k in range(2):
                for gi, pr in enumerate(grp):
                    off = 0
                    for (l, n0, n1) in pr:
                        nc.tensor.matmul(
                            pts[gi][:, off : off + (n1 - n0)],
                            lhs[k][:],
                            f2_lv[l][k][:, n0:n1],
                            start=(k == 0),
                            stop=(k == 1),
                        )
                        off += n1 - n0
            for gi, pr in enumerate(grp):
                # drain pair; split only when levels (scales) differ
                segs = []
                for (l, n0, n1) in pr:
                    if segs and segs[-1][0] == l:
                        segs[-1] = (l, segs[-1][1], segs[-1][2] + (n1 - n0))
                    else:
                        segs.append((l, n0, n1 - n0))
                off = 0
                for (l, n0, w) in segs:
                    scale_l = (1.0 / 16.0) * (0.25 ** l)
                    dst = cv_sb[:, LVL_OFF[l] + n0 : LVL_OFF[l] + n0 + w]
                    # scalar-heavy split: vector is the busier engine
                    if parity % 4 == 0:
                        nc.vector.tensor_scalar_mul(dst[:], pts[gi][:, off : off + w], scale_l)
                    else:
                        nc.scalar.mul(dst[:], pts[gi][:, off : off + w], scale_l)
                    parity += 1
                    off += w
        # store cv to DRAM (bf16): L0, then L1-3
        cvq = cvb[b][PADB : PADB + 128 * LVLSUM].rearrange("(p s) -> p s", s=LVLSUM)
        nc.sync.dma_start(out=cvq[:, 0 : LVL_N[0]], in_=cv_sb[:, 0 : LVL_N[0]])
        nc.sync.dma_start(out=cvq[:, LVL_N[0] : LVLSUM], in_=cv_sb[:, LVL_N[0] : LVLSUM])

    def emit_block_gather(b):
        cv2d = cvb[b][:].rearrange("(a x) -> a x", x=1024)
        runs = []
        for l in range(NUM_LEVELS):
            rt = runp.tile([128, RUN_L[l]], bf16, name=f"runs{l}", tag=f"runs{l}")
            nc.gpsimd.indirect_dma_start(
                out=rt[:],
                out_offset=None,
                in_=cv2d,
                in_offset=bass.IndirectOffsetOnAxis(
                    ap=idx_i[:, b, l].unsqueeze(1), axis=1
                ),
            )
            runs.append(rt)
        return runs

    def emit_block_combine(b, runs):
        # x-stage: tx[p,l,r,k] = run[p, r*Wl+k]*wx0[p,l,k] + run[p, r*Wl+k+1]*wx1[p,l,k]
        tx = cmb.tile([128, NUM_LEVELS, PS, KK], f32, name="tx", tag="tx")
        tx2 = cmb.tile([128, NUM_LEVELS, PS, KK], f32, name="tx2", tag="tx2")
        for l in range(NUM_LEVELS):
            ra = runs[l][:]
            pstep = ra.ap[0][0]
            v0 = bass.AP(ra.tensor, 0, [[pstep, 128], [LVL_W[l], PS], [1, KK]])
            v1 = bass.AP(ra.tensor, 1, [[pstep, 128], [LVL_W[l], PS], [1, KK]])
            bshape = [128, PS, KK]
            nc.vector.tensor_tensor(
                tx[:, l], v0, wx0e[:, b, l, :].unsqueeze(1).to_broadcast(bshape),
                op=Alu.mult,
            )
            nc.vector.tensor_tensor(
                tx2[:, l], v1, wx1e[:, b, l, :].unsqueeze(1).to_broadcast(bshape),
                op=Alu.mult,
            )
            nc.vector.tensor_tensor(tx[:, l], tx[:, l], tx2[:, l], op=Alu.add)
        # y-stage (fused across levels; on gpsimd to unload vector)
        outq = cmb.tile([128, NUM_LEVELS, KK, KK], f32, name="outq", tag="outq")
        outq2 = cmb.tile([128, NUM_LEVELS, KK, KK], f32, name="outq2", tag="outq2")
        bshape_y = [128, NUM_LEVELS, KK, KK]
        nc.gpsimd.tensor_tensor(
            outq[:], tx[:, :, 0:KK, :],
            wy0e[:, b].unsqueeze(3).to_broadcast(bshape_y), op=Alu.mult,
        )
        nc.gpsimd.tensor_tensor(
            outq2[:], tx[:, :, 1:PS, :],
            wy1e[:, b].unsqueeze(3).to_broadcast(bshape_y), op=Alu.mult,
        )
        nc.gpsimd.tensor_tensor(outq[:], outq[:], outq2[:], op=Alu.add)
        return outq

    def emit_block_transpose(b, outq):
        # transpose to channel-major: 3 chunks of <=128 channels
        outq_v = outq[:].rearrange("p l dy dx -> p (l dy dx)")
        for k, (c0, nk) in enumerate(OCHUNKS):
            ptt = psum_t.tile([128, 128], f32, name="ptt", tag="ptt")
            nc.tensor.transpose(
                out=ptt[:nk, :], in_=outq_v[:, c0 : c0 + nk], identity=ident[:]
            )
            if k % 2 == 0:
                nc.vector.tensor_copy(out=outT[k][0:nk, b, :], in_=ptt[:nk, :])
            else:
                nc.scalar.copy(out=outT[k][0:nk, b, :], in_=ptt[:nk, :])

    # ---------------- software pipeline over blocks (skew-2) ---------------
    # combine(b-2) is emitted at the head of iteration b: its gather finished
    # during block b-1, so neither vector nor tensor ever wait on the
    # cv-write + gather latency
    hist = []
    for b in range(NBLK):
        oq = None
        if len(hist) >= 2:
            bb, runs_bb = hist.pop(0)
            oq = emit_block_combine(bb, runs_bb)
        emit_block_gemm(b)
        runs = emit_block_gather(b)
        if oq is not None:
            emit_block_transpose(bb, oq)
        hist.append((b, runs))
    for bb, runs_bb in hist:
        oq = emit_block_combine(bb, runs_bb)
        emit_block_transpose(bb, oq)

    for k, (c0, nk) in enumerate(OCHUNKS):
        nc.sync.dma_start(out=out_ext[c0 : c0 + nk, :, :], in_=outT[k][0:nk, :, :])

    if dbg is not None:
        for l in range(NUM_LEVELS):
            rt0 = runp.tile([128, RUN_L[l]], bf16, name=f"dbgruns{l}", tag=f"runs{l}")
            cv2d0 = cvb[0][:].rearrange("(a x) -> a x", x=1024)
            nc.gpsimd.indirect_dma_start(
                out=rt0[:], out_offset=None, in_=cv2d0,
                in_offset=bass.IndirectOffsetOnAxis(
                    ap=idx_i[:, 0, l].unsqueeze(1), axis=1
                ),
            )
            nc.sync.dma_start(out=dbg[f"runs{l}"][:], in_=rt0[:])
        nc.sync.dma_start(
            out=dbg["idx"][:], in_=idx_i[:].rearrange("p b l -> p (b l)")
        )
        nc.sync.dma_start(
            out=dbg["wx0"][:], in_=wx0e[:].rearrange("p b l k -> p (b l k)")
        )
        nc.sync.dma_start(
            out=dbg["wy0"][:], in_=wy0e[:].rearrange("p b l k -> p (b l k)")
        )
        nc.sync.dma_start(
            out=dbg["cv"][:],
            in_=cvb[0][0 : PADB + 2 * LVLSUM].rearrange("(p x) -> p x", p=128),
        )


def build_program(debug=False):
    """Build (once) the single-core SPMD bass program."""
    key = ("nc", debug)
    if key in _CACHE:
        return _CACHE[key]
    import concourse.tile as tile
    import concourse.mybir as mybir
    from concourse import bacc

    f32 = mybir.dt.float32
    bf16 = mybir.dt.bfloat16
    i32 = mybir.dt.int32
    nc = bacc.Bacc(
        "TRN2",
        target_bir_lowering=False,
        debug=False,
        enable_asserts=True,
        num_devices=NCORES,
    )
    f1c = nc.dram_tensor("f1c", [D, QPC], bf16, kind="ExternalInput").ap()
    f2 = nc.dram_tensor("f2", [D, H * W], bf16, kind="ExternalInput").ap()
    crd = nc.dram_tensor("crd", [2, QPC], f32, kind="ExternalInput").ap()
    out = nc.dram_tensor("out", [NCH, H // NCORES, W], f32, kind="ExternalOutput").ap()
    dbg = None
    if debug:
        dbg = {
            "idx": nc.dram_tensor(
                "dbg_idx", [128, NBLK * NUM_LEVELS], i32, kind="ExternalOutput"
            ).ap(),
            "wx0": nc.dram_tensor(
                "dbg_wx0", [128, NBLK * NUM_LEVELS * KK], f32, kind="ExternalOutput"
            ).ap(),
            "wy0": nc.dram_tensor(
                "dbg_wy0", [128, NBLK * NUM_LEVELS * KK], f32, kind="ExternalOutput"
            ).ap(),
            "cv": nc.dram_tensor(
                "dbg_cv", [128, (PADB + 2 * LVLSUM) // 128], bf16,
                kind="ExternalOutput",
            ).ap(),
        }
        for l in range(NUM_LEVELS):
            dbg[f"runs{l}"] = nc.dram_tensor(
                f"dbg_runs{l}", [128, RUN_L[l]], bf16, kind="ExternalOutput"
            ).ap()

    from contextlib import ExitStack

    with tile.TileContext(nc) as tc, ExitStack() as ctx:
        _emit(ctx, tc, out, f1c, f2, crd, dbg=dbg)
    nc.compile()
    _CACHE[key] = nc
    return nc


def make_in_maps(fmap1, fmap2, coords):
    import concourse.mybir as mybir

    np_bf16 = mybir.dt.np(mybir.dt.bfloat16)
    f1 = np.asarray(fmap1, dtype=np.float32).reshape(D, H * W).astype(np_bf16)
    f2 = np.asarray(fmap2, dtype=np.float32).reshape(D, H * W).astype(np_bf16)
    crd = np.asarray(coords, dtype=np.float32).reshape(2, H * W)
    in_maps = []
    for c in range(NCORES):
        sl = slice(c * QPC, (c + 1) * QPC)
        in_maps.append(
            {
                "f1c": np.ascontiguousarray(f1[:, sl]),
                "f2": f2,
                "crd": np.ascontiguousarray(crd[:, sl]),
            }
        )
    return in_maps


def kernel(fmap1, fmap2, coords):
    from concourse.bass_utils import run_bass_kernel_spmd

    nc = build_program()
    in_maps = make_in_maps(fmap1, fmap2, coords)
    res = run_bass_kernel_spmd(nc, in_maps, list(range(NCORES)))
    parts = [res.results[c]["out"] for c in range(NCORES)]  # [324, 8, 128] each
    full = np.concatenate(parts, axis=1)  # [324, 64, 128]
    return full[None].astype(np.float32)
